# revision 14
# baseline (speedup 1.0000x reference)
"""Trainium2 Bass kernel for BERT self-attention.

Problem: B=16, S=512, H=1024, 16 heads x 64. Data-parallel over batch:
each of the 8 cores owns 2 batches and runs the full attention for them.

Per-core layout (T = 2*512 = 1024 local tokens), all-bf16 matmuls
(fp32 PSUM accumulation; validated max rel err ~6e-3 vs 2e-2 budget):
  - xT  [H=1024, T=1024] bf16 : hidden states transposed (host-side)
  - wqT/wkT/wvT [H, O] bf16   : weights transposed (host-side)
  - QT, KT computed as [O, T] bf16 (transposed): bias per-partition,
    added by the ACT evacuation.
  - V computed natural [T, O], stored interleaved as [128, 16*(64+1)]
    bf16 with a ones-column per head; the ones-column turns the softmax
    denominator into one extra row of the context matmul.
  - attention mask folded in as a row-scaling of V' by exp(mask/8).
  - bv folded into the final output add (softmax rows sum to 1).
  - scoresT [keys, queries] per (b, h) with head pairs row-tiled into
    disjoint PE row groups (concurrent); exp on ScalarE; ctxT' =
    V'.T @ expT; PE-transpose back to [queries, 64+1]; DVE: reciprocal
    of denom col, multiply, add bv; fine-grained DMA out.

Schedule: V projection first as an 8-bank PSUM wavefront (k-outer) so
the PE starts as soon as the first x/wv chunks land; then Q/K
projections software-pipelined one head-pair ahead of attention so the
ScalarE exp work hides under PE matmuls and the PE never stalls on the
scores->exp->ctx chain.
"""

import os
import sys

import numpy as np

if "/opt/trn_rl_repo" not in sys.path:
    sys.path.insert(0, "/opt/trn_rl_repo")

NCORES = 8
B = 16
S = 512
H = 1024
NH = 16
HS = 64
B_LOC = B // NCORES          # 2 batches per core
T = B_LOC * S                # 1024 tokens per core
NK = H // 128                # 8 contraction chunks

_prog_cache = {}
last_results = None          # BassKernelResults from the most recent run


def _ensure_ntff_hook():
    """Install antenv.axon_hooks if the image lacks it (profiling only)."""
    try:
        import antenv.axon_hooks  # noqa: F401
        return
    except ImportError:
        pass
    try:
        import types
        import antenv
        from trn_agent_boot.trn_boot import _ntff_profile_via_ctypes

        mod = types.ModuleType("antenv.axon_hooks")
        state = {"hook": None}
        mod.set_axon_ntff_profile_hook = lambda h: state.__setitem__("hook", h)
        mod.get_axon_ntff_profile_hook = lambda: state["hook"]
        sys.modules["antenv.axon_hooks"] = mod
        antenv.axon_hooks = mod
        hook = _ntff_profile_via_ctypes("/opt/axon/libaxon_pjrt.so")
        if hook is not None:
            mod.set_axon_ntff_profile_hook(hook)
    except Exception as e:  # profiling is best-effort
        print(f"ntff hook install failed: {e}", file=sys.stderr)


def _build_program():
    from concourse import bacc, mybir, tile
    import concourse.bass as bass

    f32 = mybir.dt.float32
    bf = mybir.dt.bfloat16
    Exp = mybir.ActivationFunctionType.Exp
    Ident = mybir.ActivationFunctionType.Identity

    nc = bacc.Bacc("TRN2", target_bir_lowering=False, debug=False,
                   enable_asserts=False)

    xT_d = nc.dram_tensor("xT", [H, T], bf, kind="ExternalInput").ap()
    wqT_d = nc.dram_tensor("wqT", [H, H], bf, kind="ExternalInput").ap()
    wkT_d = nc.dram_tensor("wkT", [H, H], bf, kind="ExternalInput").ap()
    wvT_d = nc.dram_tensor("wvT", [H, H], bf, kind="ExternalInput").ap()
    bq_d = nc.dram_tensor("bq2", [128, NK], f32, kind="ExternalInput").ap()
    bk_d = nc.dram_tensor("bk2", [128, NK], f32, kind="ExternalInput").ap()
    bvb_d = nc.dram_tensor("bvb", [128, H], f32, kind="ExternalInput").ap()
    maskw_d = nc.dram_tensor("maskw", [128, NK], f32, kind="ExternalInput").ap()
    ident_d = nc.dram_tensor("ident", [128, 128], f32, kind="ExternalInput").ap()
    out_d = nc.dram_tensor("out", [T, H], f32, kind="ExternalOutput").ap()

    with tile.TileContext(nc) as tc:
        with (
            tc.tile_pool(name="const", bufs=1) as const_pool,
            tc.tile_pool(name="persist", bufs=1) as persist,
            tc.tile_pool(name="outp", bufs=1) as outp,
            tc.tile_pool(name="xw", bufs=1) as xw_pool,
        ):
            # constants
            ident_sb = const_pool.tile([128, 128], f32, name="ident_sb")
            bq_sb = const_pool.tile([128, NK], f32, name="bq_sb")
            bk_sb = const_pool.tile([128, NK], f32, name="bk_sb")
            bvb_sb = const_pool.tile([128, H], f32, name="bvb_sb")
            maskw_sb = const_pool.tile([128, NK], f32, name="maskw_sb")
            ident_bf = const_pool.tile([128, 128], bf, name="ident_bf")

            # activations + weights, all resident (bf16).  Two HWDGE rings
            # (sync + scalar) stream concurrently; each ring is FIFO, so
            # order by first-use time: the x/wv chunk pairs land first
            # (V projection runs first), weights next, late-use constants
            # last.  Out-DMAs stay on the sync ring only -- a DMA on the
            # scalar ring would head-of-line block the exp stream.
            xts = [xw_pool.tile([128, T], bf, name=f"xt{k}", tag=f"xt{k}")
                   for k in range(NK)]
            wv_t = [xw_pool.tile([128, H], bf, name=f"wv{k}", tag=f"wv{k}")
                    for k in range(NK)]
            wq_t = [xw_pool.tile([128, H], bf, name=f"wq{k}", tag=f"wq{k}")
                    for k in range(NK)]
            wk_t = [xw_pool.tile([128, H], bf, name=f"wk{k}", tag=f"wk{k}")
                    for k in range(NK)]
            # Scalar ring carries ONLY the early maskw + wv loads: anything
            # more would head-of-line block the ScalarE compute stream
            # (V evacuations) behind DMA dispatches.
            for k in range(NK):
                nc.sync.dma_start(xts[k][:], xT_d[k * 128:(k + 1) * 128, :])
                nc.scalar.dma_start(wv_t[k][:], wvT_d[k * 128:(k + 1) * 128, :])
                if k == 0:
                    nc.scalar.dma_start(maskw_sb[:], maskw_d[:])
            for k in range(NK):
                nc.sync.dma_start(wq_t[k][:], wqT_d[k * 128:(k + 1) * 128, :])
            for k in range(NK):
                nc.sync.dma_start(wk_t[k][:], wkT_d[k * 128:(k + 1) * 128, :])
            nc.sync.dma_start(ident_sb[:], ident_d[:])
            nc.vector.tensor_copy(ident_bf[:], ident_sb[:])
            nc.sync.dma_start(bq_sb[:], bq_d[:])
            nc.sync.dma_start(bk_sb[:], bk_d[:])
            nc.sync.dma_start(bvb_sb[:], bvb_d[:])

            qt_sb = [persist.tile([128, T], bf, name=f"qt{i}", tag=f"qt{i}")
                     for i in range(NK)]
            kt_sb = [persist.tile([128, T], bf, name=f"kt{i}", tag=f"kt{i}")
                     for i in range(NK)]
            # V' tiles: [128, 16 heads * 65]; col 64 of each head = ones*w
            vp_sb = [persist.tile([128, NH * (HS + 1)], bf, name=f"vp{i}",
                                  tag=f"vp{i}")
                     for i in range(NK)]
            ot_sb = [outp.tile([128, H], f32, name=f"ot{i}", tag=f"ot{i}")
                     for i in range(NK)]

            # ---- PE warm-up: dummy matmuls on a memset tile while the
            # first x/wv chunks stream in.  The HAM clock gate defaults the
            # PE to 1.2 GHz and only releases to 2.4 GHz after ~3.4us of
            # sustained activity; burning the DMA window on throwaway
            # matmuls means every real matmul runs at full clock.
            warm_sb = const_pool.tile([128, 512], bf, name="warm_sb")
            nc.vector.memset(warm_sb[:], 0.0)

            # ---- V projection: natural [t, o] into interleaved V'.
            # Wave A (8 groups, k-outer): every arriving (x, wv) chunk pair
            # immediately unlocks 8 matmuls, so the PE is DMA-paced during
            # the initial load.  Wave B (groups resident by then) runs
            # group-sequential so completions stagger and the ScalarE
            # evacuations overlap compute instead of bunching at the end.
            def v_evac(pss_g, tt, oh):
                vv = vp_sb[tt].rearrange("p (h e) -> p h e", e=HS + 1)
                nc.scalar.activation(
                    vv[:, oh * 8:(oh + 1) * 8, 0:HS],
                    pss_g.rearrange("p (h d) -> p h d", d=HS),
                    mybir.ActivationFunctionType.Identity,
                    scale=maskw_sb[:, tt:tt + 1])

            with tc.tile_pool(name="pwarm", bufs=1, space="PSUM") as pwarm:
                ps_w = pwarm.tile([128, 512], f32, name="ps_w")
                for _ in range(26):
                    nc.tensor.matmul(ps_w[:], warm_sb[:, 0:128],
                                     warm_sb[:], start=True, stop=True)

            with tc.tile_pool(name="pv", bufs=8, space="PSUM") as pv:
                groups = [(tt, oh) for tt in range(4) for oh in range(2)]
                pss = [pv.tile([128, 512], f32, name=f"pv{gi}", tag="pv")
                       for gi in range(8)]
                for k in range(NK):
                    for gi, (tt, oh) in enumerate(groups):
                        nc.tensor.matmul(
                            pss[gi][:],
                            xts[k][:, tt * 128:(tt + 1) * 128],
                            wv_t[k][:, oh * 512:(oh + 1) * 512],
                            start=(k == 0), stop=(k == NK - 1),
                        )
                for gi, (tt, oh) in enumerate(groups):
                    v_evac(pss[gi], tt, oh)
                for tt in range(4, NK):
                    for oh in range(2):
                        ps = pv.tile([128, 512], f32, name="pvb", tag="pv")
                        for k in range(NK):
                            nc.tensor.matmul(
                                ps[:],
                                xts[k][:, tt * 128:(tt + 1) * 128],
                                wv_t[k][:, oh * 512:(oh + 1) * 512],
                                start=(k == 0), stop=(k == NK - 1),
                            )
                        v_evac(ps, tt, oh)
                for tt in range(NK):
                    vv = vp_sb[tt].rearrange("p (h e) -> p h e", e=HS + 1)
                    nc.vector.tensor_copy(
                        vv[:, :, HS:HS + 1],
                        maskw_sb[:, tt:tt + 1].broadcast_to([128, NH, 1]))

            # ---- Q/K projections software-pipelined with attention ----
            with (
                tc.tile_pool(name="pproj", bufs=2, space="PSUM") as pproj,
                tc.tile_pool(name="psc", bufs=2, space="PSUM") as sc_pool,
                tc.tile_pool(name="pcx", bufs=1, space="PSUM") as cx_pool,
                tc.tile_pool(name="ptr", bufs=1, space="PSUM") as tr_pool,
                tc.tile_pool(name="ex", bufs=6) as ex_pool,
                tc.tile_pool(name="cs", bufs=4) as cs_pool,
                tc.tile_pool(name="rc", bufs=4) as rc_pool,
            ):
                def proj_group(w_t, dst, bias_sb, hp, th, on_dve):
                    """One [128, 512] projection PSUM group.  Q evacuates
                    on DVE, K on ScalarE, to balance the two engines (the
                    ScalarE is near-saturated with exp)."""
                    ps = pproj.tile([128, 512], f32, name="pp", tag="pp")
                    for k in range(NK):
                        nc.tensor.matmul(
                            ps[:],
                            w_t[k][:, hp * 128:(hp + 1) * 128],
                            xts[k][:, th * 512:(th + 1) * 512],
                            start=(k == 0), stop=(k == NK - 1),
                        )
                    if on_dve:
                        nc.vector.tensor_scalar(
                            dst[hp][:, th * 512:(th + 1) * 512], ps[:],
                            bias_sb[:, hp:hp + 1], None,
                            mybir.AluOpType.add)
                    else:
                        nc.scalar.activation(
                            dst[hp][:, th * 512:(th + 1) * 512], ps[:],
                            Ident, bias=bias_sb[:, hp:hp + 1])

                def emit_scores(hp, b, half, exs):
                    """Scores for head pair hp, batch b, key-half `half`.

                    Two K=64 matmuls land in disjoint PE row groups and run
                    concurrently; exp (scale 1/8) evacuates on ScalarE to
                    bf16 ex."""
                    pair = (2 * hp, 2 * hp + 1)
                    scs = {h: sc_pool.tile([128, 1024], f32, name="sc",
                                           tag="sc")
                           for h in pair}
                    for j in range(2):
                        kt = half * 2 + j
                        c0 = b * 512 + kt * 128
                        for h in pair:
                            hb = (h % 2) * HS
                            nc.tensor.matmul(
                                scs[h][:, j * 512:(j + 1) * 512],
                                kt_sb[hp][hb:hb + HS, c0:c0 + 128],
                                qt_sb[hp][hb:hb + HS,
                                          b * 512:(b + 1) * 512],
                                start=True, stop=True,
                            )
                    for h in pair:
                        nc.scalar.activation(
                            exs[(b, h)][:, half * 1024:(half + 1) * 1024],
                            scs[h][:], Exp, scale=0.125)

                def emit_ctx(hp, b, h, exs, dma_out=False):
                    """ctxT' = V'.T @ expT -> [65, 512] (row 64 = denom);
                    PE-transpose to [queries, 65]; DVE: reciprocal,
                    scale + bias into ot_sb.  With dma_out (second head of
                    the pair) each qt tile's [128, 128] output slice DMAs
                    out right after its STT, overlapping the epilogue."""
                    ex = exs[(b, h)]
                    cx = cx_pool.tile([HS + 1, 512], f32, name="cx", tag="cx")
                    for kt in range(4):
                        vv = vp_sb[b * 4 + kt].rearrange(
                            "p (h e) -> p h e", e=HS + 1)
                        nc.tensor.matmul(
                            cx[:],
                            vv[:, h, :],
                            ex[:, kt * 512:(kt + 1) * 512],
                            start=(kt == 0), stop=(kt == 3),
                        )
                    cs = cs_pool.tile([HS + 1, 512], bf, name="cs", tag="cs")
                    nc.vector.tensor_copy(cs[:], cx[:])
                    tr = tr_pool.tile([128, 4 * (HS + 2)], bf,
                                      name="tr", tag="tr")
                    trv = tr.rearrange("p (q e) -> p q e", e=HS + 2)
                    for qt in range(4):
                        nc.tensor.transpose(
                            trv[:, qt, 0:HS + 1],
                            cs[:, qt * 128:(qt + 1) * 128],
                            ident_bf[0:HS + 1, 0:HS + 1])
                    rc = rc_pool.tile([128, 4, 1], f32, name="rc", tag="rc")
                    nc.vector.reciprocal(rc[:], trv[:, :, HS:HS + 1])
                    for qt in range(4):
                        osl = ot_sb[b * 4 + qt][:, h * HS:(h + 1) * HS]
                        # out = (ctx * 1/denom) + bv  in one DVE op
                        nc.vector.scalar_tensor_tensor(
                            osl, trv[:, qt, 0:HS], rc[:, qt, :],
                            bvb_sb[:, h * HS:(h + 1) * HS],
                            mybir.AluOpType.mult, mybir.AluOpType.add)
                        if dma_out:
                            r0 = (b * 4 + qt) * 128
                            nc.sync.dma_start(
                                out_d[r0:r0 + 128, hp * 128:(hp + 1) * 128],
                                ot_sb[b * 4 + qt][:, hp * 128:(hp + 1) * 128])

                # prologue: head pair 0's Q/K projections
                for th in range(2):
                    proj_group(wq_t, qt_sb, bq_sb, 0, th, on_dve=True)
                for th in range(2):
                    proj_group(wk_t, kt_sb, bk_sb, 0, th, on_dve=False)

                for hp in range(NH // 2):
                    nxt = hp + 1 if hp + 1 < NH // 2 else None
                    exs = {(b, h): ex_pool.tile([128, 2048], bf, name="ex",
                                                tag="ex")
                           for b in range(B_LOC)
                           for h in (2 * hp, 2 * hp + 1)}
                    # interleave next pair's projections between attention
                    # stages: proj matmuls keep the PE busy while ScalarE
                    # drains exp and DVE drains the ctx epilogue.  The th0
                    # groups go first: hp+1's first scores (batch 0) read
                    # only the th0 halves of Q/K, so their evacuations must
                    # land early in the ACT/DVE queues to avoid a stall at
                    # the hp boundary.
                    emit_scores(hp, 0, 0, exs)
                    if nxt is not None:
                        proj_group(wk_t, kt_sb, bk_sb, nxt, 0, on_dve=False)
                    emit_scores(hp, 0, 1, exs)
                    if nxt is not None:
                        proj_group(wq_t, qt_sb, bq_sb, nxt, 0, on_dve=True)
                    emit_scores(hp, 1, 0, exs)
                    emit_ctx(hp, 0, 2 * hp, exs)
                    emit_scores(hp, 1, 1, exs)
                    if nxt is not None:
                        proj_group(wk_t, kt_sb, bk_sb, nxt, 1, on_dve=False)
                    emit_ctx(hp, 0, 2 * hp + 1, exs, dma_out=True)
                    if nxt is not None:
                        proj_group(wq_t, qt_sb, bq_sb, nxt, 1, on_dve=True)
                    emit_ctx(hp, 1, 2 * hp, exs)
                    emit_ctx(hp, 1, 2 * hp + 1, exs, dma_out=True)

    nc.compile()
    return nc


def _get_program():
    if "nc" not in _prog_cache:
        _prog_cache["nc"] = _build_program()
    return _prog_cache["nc"]


def kernel(hidden_states, attention_mask, Wq, bq, Wk, bk, Wv, bv):
    global last_results
    import ml_dtypes
    from concourse import bass_utils

    bf16 = ml_dtypes.bfloat16

    hidden_states = np.ascontiguousarray(np.asarray(hidden_states,
                                                    dtype=np.float32))
    attention_mask = np.asarray(attention_mask, dtype=np.float32)
    Wq = np.asarray(Wq, dtype=np.float32)
    Wk = np.asarray(Wk, dtype=np.float32)
    Wv = np.asarray(Wv, dtype=np.float32)
    bq = np.asarray(bq, dtype=np.float32)
    bk = np.asarray(bk, dtype=np.float32)
    bv = np.asarray(bv, dtype=np.float32)

    nc = _get_program()

    wqT = np.ascontiguousarray(Wq.T.astype(bf16))
    wkT = np.ascontiguousarray(Wk.T.astype(bf16))
    wvT = np.ascontiguousarray(Wv.T.astype(bf16))
    bq2 = np.ascontiguousarray(bq.reshape(NK, 128).T)
    bk2 = np.ascontiguousarray(bk.reshape(NK, 128).T)
    bvb = np.ascontiguousarray(np.tile(bv[None, :], (128, 1)))
    ident = np.eye(128, dtype=np.float32)

    mask = attention_mask.reshape(B, S)

    in_maps = []
    for c in range(NCORES):
        xT = np.ascontiguousarray(
            hidden_states[c * B_LOC:(c + 1) * B_LOC].reshape(T, H).T
            .astype(bf16))
        # maskw[p, b*4+kt] = exp(mask[b, kt*128+p] / 8)
        mw = np.exp(mask[c * B_LOC:(c + 1) * B_LOC].reshape(B_LOC, 4, 128)
                    / 8.0).transpose(2, 0, 1).reshape(128, NK)
        in_maps.append({
            "xT": xT,
            "wqT": wqT, "wkT": wkT, "wvT": wvT,
            "bq2": bq2, "bk2": bk2,
            "bvb": bvb,
            "maskw": np.ascontiguousarray(mw.astype(np.float32)),
            "ident": ident,
        })

    trace = bool(os.environ.get("BASS_TRACE"))
    if trace:
        _ensure_ntff_hook()
    res = bass_utils.run_bass_kernel_spmd(
        nc, in_maps, core_ids=list(range(NCORES)), trace=trace,
    )
    last_results = res

    out = np.empty((B, S, H), dtype=np.float32)
    for c in range(NCORES):
        oc = res.results[c]["out"]
        out[c * B_LOC:(c + 1) * B_LOC] = oc.reshape(B_LOC, S, H)
    return out


# revision 15
# speedup vs baseline: 1.0163x; 1.0163x over previous
"""Trainium2 Bass kernel for BERT self-attention.

Problem: B=16, S=512, H=1024, 16 heads x 64. Data-parallel over batch:
each of the 8 cores owns 2 batches and runs the full attention for them.

Per-core layout (T = 2*512 = 1024 local tokens), all-bf16 matmuls
(fp32 PSUM accumulation; validated max rel err ~6e-3 vs 2e-2 budget):
  - xT  [H=1024, T=1024] bf16 : hidden states transposed (host-side)
  - wqT/wkT/wvT [H, O] bf16   : weights transposed (host-side)
  - QT, KT computed as [O, T] bf16 (transposed): bias per-partition,
    added by the ACT evacuation.
  - V computed natural [T, O], stored interleaved as [128, 16*(64+1)]
    bf16 with a ones-column per head; the ones-column turns the softmax
    denominator into one extra row of the context matmul.
  - attention mask folded in as a row-scaling of V' by exp(mask/8).
  - bv folded into the final output add (softmax rows sum to 1).
  - scoresT [keys, queries] per (b, h) with head pairs row-tiled into
    disjoint PE row groups (concurrent); exp on ScalarE; ctxT' =
    V'.T @ expT; PE-transpose back to [queries, 64+1]; DVE: reciprocal
    of denom col, multiply, add bv; fine-grained DMA out.

Schedule: V projection first as an 8-bank PSUM wavefront (k-outer) so
the PE starts as soon as the first x/wv chunks land; then Q/K
projections software-pipelined one head-pair ahead of attention so the
ScalarE exp work hides under PE matmuls and the PE never stalls on the
scores->exp->ctx chain.
"""

import os
import sys

import numpy as np

if "/opt/trn_rl_repo" not in sys.path:
    sys.path.insert(0, "/opt/trn_rl_repo")

NCORES = 8
B = 16
S = 512
H = 1024
NH = 16
HS = 64
B_LOC = B // NCORES          # 2 batches per core
T = B_LOC * S                # 1024 tokens per core
NK = H // 128                # 8 contraction chunks

_prog_cache = {}
last_results = None          # BassKernelResults from the most recent run


def _ensure_ntff_hook():
    """Install antenv.axon_hooks if the image lacks it (profiling only)."""
    try:
        import antenv.axon_hooks  # noqa: F401
        return
    except ImportError:
        pass
    try:
        import types
        import antenv
        from trn_agent_boot.trn_boot import _ntff_profile_via_ctypes

        mod = types.ModuleType("antenv.axon_hooks")
        state = {"hook": None}
        mod.set_axon_ntff_profile_hook = lambda h: state.__setitem__("hook", h)
        mod.get_axon_ntff_profile_hook = lambda: state["hook"]
        sys.modules["antenv.axon_hooks"] = mod
        antenv.axon_hooks = mod
        hook = _ntff_profile_via_ctypes("/opt/axon/libaxon_pjrt.so")
        if hook is not None:
            mod.set_axon_ntff_profile_hook(hook)
    except Exception as e:  # profiling is best-effort
        print(f"ntff hook install failed: {e}", file=sys.stderr)


def _build_program():
    from concourse import bacc, mybir, tile
    import concourse.bass as bass

    f32 = mybir.dt.float32
    bf = mybir.dt.bfloat16
    Exp = mybir.ActivationFunctionType.Exp
    Ident = mybir.ActivationFunctionType.Identity

    nc = bacc.Bacc("TRN2", target_bir_lowering=False, debug=False,
                   enable_asserts=False)

    xT_d = nc.dram_tensor("xT", [H, T], bf, kind="ExternalInput").ap()
    wqT_d = nc.dram_tensor("wqT", [H, H], bf, kind="ExternalInput").ap()
    wkT_d = nc.dram_tensor("wkT", [H, H], bf, kind="ExternalInput").ap()
    wvT_d = nc.dram_tensor("wvT", [H, H], bf, kind="ExternalInput").ap()
    bq_d = nc.dram_tensor("bq2", [128, NK], f32, kind="ExternalInput").ap()
    bk_d = nc.dram_tensor("bk2", [128, NK], f32, kind="ExternalInput").ap()
    bvb_d = nc.dram_tensor("bvb", [128, H], f32, kind="ExternalInput").ap()
    maskw_d = nc.dram_tensor("maskw", [128, NK], f32, kind="ExternalInput").ap()
    ident_d = nc.dram_tensor("ident", [128, 128], f32, kind="ExternalInput").ap()
    out_d = nc.dram_tensor("out", [T, H], f32, kind="ExternalOutput").ap()

    with tile.TileContext(nc) as tc:
        with (
            tc.tile_pool(name="const", bufs=1) as const_pool,
            tc.tile_pool(name="persist", bufs=1) as persist,
            tc.tile_pool(name="outp", bufs=1) as outp,
            tc.tile_pool(name="xw", bufs=1) as xw_pool,
        ):
            # constants
            ident_sb = const_pool.tile([128, 128], f32, name="ident_sb")
            bq_sb = const_pool.tile([128, NK], f32, name="bq_sb")
            bk_sb = const_pool.tile([128, NK], f32, name="bk_sb")
            bvb_sb = const_pool.tile([128, H], f32, name="bvb_sb")
            maskw_sb = const_pool.tile([128, NK], f32, name="maskw_sb")
            ident_bf = const_pool.tile([128, 128], bf, name="ident_bf")

            # activations + weights, all resident (bf16).  Two HWDGE rings
            # (sync + scalar) stream concurrently; each ring is FIFO, so
            # order by first-use time: the x/wv chunk pairs land first
            # (V projection runs first), weights next, late-use constants
            # last.  Out-DMAs stay on the sync ring only -- a DMA on the
            # scalar ring would head-of-line block the exp stream.
            xts = [xw_pool.tile([128, T], bf, name=f"xt{k}", tag=f"xt{k}")
                   for k in range(NK)]
            wv_t = [xw_pool.tile([128, H], bf, name=f"wv{k}", tag=f"wv{k}")
                    for k in range(NK)]
            wq_t = [xw_pool.tile([128, H], bf, name=f"wq{k}", tag=f"wq{k}")
                    for k in range(NK)]
            wk_t = [xw_pool.tile([128, H], bf, name=f"wk{k}", tag=f"wk{k}")
                    for k in range(NK)]
            # Scalar ring carries ONLY the early maskw + wv loads: anything
            # more would head-of-line block the ScalarE compute stream
            # (V evacuations) behind DMA dispatches.
            for k in range(NK):
                nc.sync.dma_start(xts[k][:], xT_d[k * 128:(k + 1) * 128, :])
                nc.scalar.dma_start(wv_t[k][:], wvT_d[k * 128:(k + 1) * 128, :])
                if k == 0:
                    nc.scalar.dma_start(maskw_sb[:], maskw_d[:])
            for k in range(NK):
                nc.sync.dma_start(wq_t[k][:], wqT_d[k * 128:(k + 1) * 128, :])
            for k in range(NK):
                nc.sync.dma_start(wk_t[k][:], wkT_d[k * 128:(k + 1) * 128, :])
            nc.sync.dma_start(ident_sb[:], ident_d[:])
            nc.vector.tensor_copy(ident_bf[:], ident_sb[:])
            nc.sync.dma_start(bq_sb[:], bq_d[:])
            nc.sync.dma_start(bk_sb[:], bk_d[:])
            nc.sync.dma_start(bvb_sb[:], bvb_d[:])

            qt_sb = [persist.tile([128, T], bf, name=f"qt{i}", tag=f"qt{i}")
                     for i in range(NK)]
            kt_sb = [persist.tile([128, T], bf, name=f"kt{i}", tag=f"kt{i}")
                     for i in range(NK)]
            # V' tiles: [128, 16 heads * 65]; col 64 of each head = ones*w
            vp_sb = [persist.tile([128, NH * (HS + 1)], bf, name=f"vp{i}",
                                  tag=f"vp{i}")
                     for i in range(NK)]
            ot_sb = [outp.tile([128, H], f32, name=f"ot{i}", tag=f"ot{i}")
                     for i in range(NK)]

            # ---- PE warm-up: dummy matmuls on a memset tile while the
            # first x/wv chunks stream in.  The HAM clock gate defaults the
            # PE to 1.2 GHz and only releases to 2.4 GHz after ~3.4us of
            # sustained activity; burning the DMA window on throwaway
            # matmuls means every real matmul runs at full clock.
            warm_sb = const_pool.tile([128, 512], bf, name="warm_sb")
            nc.vector.memset(warm_sb[:], 0.0)

            # ---- V projection: natural [t, o] into interleaved V'.
            # Wave A (8 groups, k-outer): every arriving (x, wv) chunk pair
            # immediately unlocks 8 matmuls, so the PE is DMA-paced during
            # the initial load.  Wave B (groups resident by then) runs
            # group-sequential so completions stagger and the ScalarE
            # evacuations overlap compute instead of bunching at the end.
            def v_evac(pss_g, tt, oh):
                vv = vp_sb[tt].rearrange("p (h e) -> p h e", e=HS + 1)
                nc.scalar.activation(
                    vv[:, oh * 8:(oh + 1) * 8, 0:HS],
                    pss_g.rearrange("p (h d) -> p h d", d=HS),
                    mybir.ActivationFunctionType.Identity,
                    scale=maskw_sb[:, tt:tt + 1])

            with tc.tile_pool(name="pwarm", bufs=1, space="PSUM") as pwarm:
                ps_w = pwarm.tile([128, 512], f32, name="ps_w")
                for _ in range(8):
                    nc.tensor.matmul(ps_w[:], warm_sb[:, 0:128],
                                     warm_sb[:], start=True, stop=True)

            with tc.tile_pool(name="pv", bufs=8, space="PSUM") as pv:
                groups = [(tt, oh) for tt in range(4) for oh in range(2)]
                pss = [pv.tile([128, 512], f32, name=f"pv{gi}", tag="pv")
                       for gi in range(8)]
                for k in range(NK):
                    for gi, (tt, oh) in enumerate(groups):
                        nc.tensor.matmul(
                            pss[gi][:],
                            xts[k][:, tt * 128:(tt + 1) * 128],
                            wv_t[k][:, oh * 512:(oh + 1) * 512],
                            start=(k == 0), stop=(k == NK - 1),
                        )
                for gi, (tt, oh) in enumerate(groups):
                    v_evac(pss[gi], tt, oh)
                for tt in range(4, NK):
                    for oh in range(2):
                        ps = pv.tile([128, 512], f32, name="pvb", tag="pv")
                        for k in range(NK):
                            nc.tensor.matmul(
                                ps[:],
                                xts[k][:, tt * 128:(tt + 1) * 128],
                                wv_t[k][:, oh * 512:(oh + 1) * 512],
                                start=(k == 0), stop=(k == NK - 1),
                            )
                        v_evac(ps, tt, oh)
                for tt in range(NK):
                    vv = vp_sb[tt].rearrange("p (h e) -> p h e", e=HS + 1)
                    nc.vector.tensor_copy(
                        vv[:, :, HS:HS + 1],
                        maskw_sb[:, tt:tt + 1].broadcast_to([128, NH, 1]))

            # ---- Q/K projections software-pipelined with attention ----
            with (
                tc.tile_pool(name="pproj", bufs=2, space="PSUM") as pproj,
                tc.tile_pool(name="psc", bufs=2, space="PSUM") as sc_pool,
                tc.tile_pool(name="pcx", bufs=1, space="PSUM") as cx_pool,
                tc.tile_pool(name="ptr", bufs=1, space="PSUM") as tr_pool,
                tc.tile_pool(name="ex", bufs=6) as ex_pool,
                tc.tile_pool(name="cs", bufs=4) as cs_pool,
                tc.tile_pool(name="rc", bufs=4) as rc_pool,
            ):
                def proj_group(w_t, dst, bias_sb, hp, th, on_dve):
                    """One [128, 512] projection PSUM group.  Q evacuates
                    on DVE, K on ScalarE, to balance the two engines (the
                    ScalarE is near-saturated with exp)."""
                    ps = pproj.tile([128, 512], f32, name="pp", tag="pp")
                    for k in range(NK):
                        nc.tensor.matmul(
                            ps[:],
                            w_t[k][:, hp * 128:(hp + 1) * 128],
                            xts[k][:, th * 512:(th + 1) * 512],
                            start=(k == 0), stop=(k == NK - 1),
                        )
                    if on_dve:
                        nc.vector.tensor_scalar(
                            dst[hp][:, th * 512:(th + 1) * 512], ps[:],
                            bias_sb[:, hp:hp + 1], None,
                            mybir.AluOpType.add)
                    else:
                        nc.scalar.activation(
                            dst[hp][:, th * 512:(th + 1) * 512], ps[:],
                            Ident, bias=bias_sb[:, hp:hp + 1])

                def emit_scores(hp, b, half, exs):
                    """Scores for head pair hp, batch b, key-half `half`.

                    Two K=64 matmuls land in disjoint PE row groups and run
                    concurrently; exp (scale 1/8) evacuates on ScalarE to
                    bf16 ex."""
                    pair = (2 * hp, 2 * hp + 1)
                    scs = {h: sc_pool.tile([128, 1024], f32, name="sc",
                                           tag="sc")
                           for h in pair}
                    for j in range(2):
                        kt = half * 2 + j
                        c0 = b * 512 + kt * 128
                        for h in pair:
                            hb = (h % 2) * HS
                            nc.tensor.matmul(
                                scs[h][:, j * 512:(j + 1) * 512],
                                kt_sb[hp][hb:hb + HS, c0:c0 + 128],
                                qt_sb[hp][hb:hb + HS,
                                          b * 512:(b + 1) * 512],
                                start=True, stop=True,
                            )
                    for h in pair:
                        nc.scalar.activation(
                            exs[(b, h)][:, half * 1024:(half + 1) * 1024],
                            scs[h][:], Exp, scale=0.125)

                def emit_ctx(hp, b, h, exs, dma_out=False):
                    """ctxT' = V'.T @ expT -> [65, 512] (row 64 = denom);
                    PE-transpose to [queries, 65]; DVE: reciprocal,
                    scale + bias into ot_sb.  With dma_out (second head of
                    the pair) each qt tile's [128, 128] output slice DMAs
                    out right after its STT, overlapping the epilogue."""
                    ex = exs[(b, h)]
                    cx = cx_pool.tile([HS + 1, 512], f32, name="cx", tag="cx")
                    for kt in range(4):
                        vv = vp_sb[b * 4 + kt].rearrange(
                            "p (h e) -> p h e", e=HS + 1)
                        nc.tensor.matmul(
                            cx[:],
                            vv[:, h, :],
                            ex[:, kt * 512:(kt + 1) * 512],
                            start=(kt == 0), stop=(kt == 3),
                        )
                    cs = cs_pool.tile([HS + 1, 512], bf, name="cs", tag="cs")
                    nc.vector.tensor_copy(cs[:], cx[:])
                    tr = tr_pool.tile([128, 4 * (HS + 2)], bf,
                                      name="tr", tag="tr")
                    trv = tr.rearrange("p (q e) -> p q e", e=HS + 2)
                    for qt in range(4):
                        nc.tensor.transpose(
                            trv[:, qt, 0:HS + 1],
                            cs[:, qt * 128:(qt + 1) * 128],
                            ident_bf[0:HS + 1, 0:HS + 1])
                    rc = rc_pool.tile([128, 4, 1], f32, name="rc", tag="rc")
                    nc.vector.reciprocal(rc[:], trv[:, :, HS:HS + 1])
                    for qt in range(4):
                        osl = ot_sb[b * 4 + qt][:, h * HS:(h + 1) * HS]
                        # out = (ctx * 1/denom) + bv  in one DVE op
                        nc.vector.scalar_tensor_tensor(
                            osl, trv[:, qt, 0:HS], rc[:, qt, :],
                            bvb_sb[:, h * HS:(h + 1) * HS],
                            mybir.AluOpType.mult, mybir.AluOpType.add)
                        if dma_out:
                            r0 = (b * 4 + qt) * 128
                            nc.sync.dma_start(
                                out_d[r0:r0 + 128, hp * 128:(hp + 1) * 128],
                                ot_sb[b * 4 + qt][:, hp * 128:(hp + 1) * 128])

                # prologue: head pair 0's Q/K projections
                for th in range(2):
                    proj_group(wq_t, qt_sb, bq_sb, 0, th, on_dve=True)
                for th in range(2):
                    proj_group(wk_t, kt_sb, bk_sb, 0, th, on_dve=False)

                for hp in range(NH // 2):
                    nxt = hp + 1 if hp + 1 < NH // 2 else None
                    exs = {(b, h): ex_pool.tile([128, 2048], bf, name="ex",
                                                tag="ex")
                           for b in range(B_LOC)
                           for h in (2 * hp, 2 * hp + 1)}
                    # interleave next pair's projections between attention
                    # stages: proj matmuls keep the PE busy while ScalarE
                    # drains exp and DVE drains the ctx epilogue.  The th0
                    # groups go first: hp+1's first scores (batch 0) read
                    # only the th0 halves of Q/K, so their evacuations must
                    # land early in the ACT/DVE queues to avoid a stall at
                    # the hp boundary.
                    emit_scores(hp, 0, 0, exs)
                    if nxt is not None:
                        proj_group(wk_t, kt_sb, bk_sb, nxt, 0, on_dve=False)
                    emit_scores(hp, 0, 1, exs)
                    if nxt is not None:
                        proj_group(wq_t, qt_sb, bq_sb, nxt, 0, on_dve=True)
                    emit_scores(hp, 1, 0, exs)
                    emit_ctx(hp, 0, 2 * hp, exs)
                    emit_scores(hp, 1, 1, exs)
                    if nxt is not None:
                        proj_group(wk_t, kt_sb, bk_sb, nxt, 1, on_dve=False)
                    emit_ctx(hp, 0, 2 * hp + 1, exs, dma_out=True)
                    if nxt is not None:
                        proj_group(wq_t, qt_sb, bq_sb, nxt, 1, on_dve=True)
                    emit_ctx(hp, 1, 2 * hp, exs)
                    emit_ctx(hp, 1, 2 * hp + 1, exs, dma_out=True)

    nc.compile()
    return nc


def _get_program():
    if "nc" not in _prog_cache:
        _prog_cache["nc"] = _build_program()
    return _prog_cache["nc"]


def kernel(hidden_states, attention_mask, Wq, bq, Wk, bk, Wv, bv):
    global last_results
    import ml_dtypes
    from concourse import bass_utils

    bf16 = ml_dtypes.bfloat16

    hidden_states = np.ascontiguousarray(np.asarray(hidden_states,
                                                    dtype=np.float32))
    attention_mask = np.asarray(attention_mask, dtype=np.float32)
    Wq = np.asarray(Wq, dtype=np.float32)
    Wk = np.asarray(Wk, dtype=np.float32)
    Wv = np.asarray(Wv, dtype=np.float32)
    bq = np.asarray(bq, dtype=np.float32)
    bk = np.asarray(bk, dtype=np.float32)
    bv = np.asarray(bv, dtype=np.float32)

    nc = _get_program()

    wqT = np.ascontiguousarray(Wq.T.astype(bf16))
    wkT = np.ascontiguousarray(Wk.T.astype(bf16))
    wvT = np.ascontiguousarray(Wv.T.astype(bf16))
    bq2 = np.ascontiguousarray(bq.reshape(NK, 128).T)
    bk2 = np.ascontiguousarray(bk.reshape(NK, 128).T)
    bvb = np.ascontiguousarray(np.tile(bv[None, :], (128, 1)))
    ident = np.eye(128, dtype=np.float32)

    mask = attention_mask.reshape(B, S)

    in_maps = []
    for c in range(NCORES):
        xT = np.ascontiguousarray(
            hidden_states[c * B_LOC:(c + 1) * B_LOC].reshape(T, H).T
            .astype(bf16))
        # maskw[p, b*4+kt] = exp(mask[b, kt*128+p] / 8)
        mw = np.exp(mask[c * B_LOC:(c + 1) * B_LOC].reshape(B_LOC, 4, 128)
                    / 8.0).transpose(2, 0, 1).reshape(128, NK)
        in_maps.append({
            "xT": xT,
            "wqT": wqT, "wkT": wkT, "wvT": wvT,
            "bq2": bq2, "bk2": bk2,
            "bvb": bvb,
            "maskw": np.ascontiguousarray(mw.astype(np.float32)),
            "ident": ident,
        })

    trace = bool(os.environ.get("BASS_TRACE"))
    if trace:
        _ensure_ntff_hook()
    res = bass_utils.run_bass_kernel_spmd(
        nc, in_maps, core_ids=list(range(NCORES)), trace=trace,
    )
    last_results = res

    out = np.empty((B, S, H), dtype=np.float32)
    for c in range(NCORES):
        oc = res.results[c]["out"]
        out[c * B_LOC:(c + 1) * B_LOC] = oc.reshape(B_LOC, S, H)
    return out
